# revision 1
# baseline (speedup 1.0000x reference)
"""CLUB mutual-information upper bound (loss_fn) on 8 Trainium2 NeuronCores.

Math: reference computes
    h  = relu(x1 @ W1 + b1); h = relu(h @ W2 + b2); g = tanh(h @ W3 + b3)
    mu, logvar = split(g); iv = exp(-logvar)
    pos = -0.5 (mu - x2)^2 iv
    neg = -0.5 mean_j[(mu_i - x2_j)^2] iv
    mi  = mean_i sum_d (pos - neg)

The O(N^2 D) pairwise term collapses with m1 = mean_j x2, m2 = mean_j x2^2:
    pos - neg = -0.5 iv [x2_i^2 - m2 - 2 mu (x2_i - m1)]
which further decomposes into per-core-local reductions (rows sharded 128/core):
    S0_d = sum_i iv          S1_d = sum_i mu*iv
    T0_d = sum_i iv*x2^2     T1_d = sum_i mu*iv*x2
    p1_d = sum_j x2          p2_d = sum_j x2^2
    N * mi = sum_d [ -0.5*T0 + 0.5*m2*S0 + T1 - m1*S1 ],  m1 = p1/N, m2 = p2/N
so each core needs ONLY its own 128-row shard of x1/x2 plus the (replicated)
weights: data-parallel, no collectives, cross-core coupling resolved on host.

Device layout is feature-major ([feature partitions, row free-axis]); the host
packs pre-transposed shards + weights into one blob so the kernel is a single
input DMA, 12 fp32 matmuls, 7 activations, 5 vector ops, 1 output DMA.
"""

import sys
from contextlib import ExitStack

import numpy as np

sys.path.insert(0, "/opt/trn_rl_repo")

import concourse.bass as bass
import concourse.tile as tile
from concourse import mybir
from concourse.bass_utils import run_bass_kernel_spmd

DT = mybir.dt.float32
NCORES = 8
N = 1024
X1D = 256
X2D = 128
HID = 256
ROWS = N // NCORES  # 128
P = 128

# blob (per-core): [128 partitions, 1926] f32
#   [0:256)     x1sT   col k*128+j   = x1s[j, k*128+p]
#   [256:384)   x2sT   col 256+j     = x2s[j, p]
#   [384:390)   biases col 384+2l+m  = b_l[m*128+p]
#   [390:1926)  W      col 390+l*512+k*256+j = W_l[k*128+p, j]
X2T_OFF = 256
B_OFF = 384
W_OFF = 390
BLOB_W = W_OFF + 3 * 512  # 1926

_module_cache = None

# split point for the two parallel input DMAs (sync ring: x-sec + W1,
# scalar ring: W2 + W3)
DMA_SPLIT = W_OFF + 512  # 902


def _build_module():
    """Raw-Bass build, hand-rolled semaphores, no Tile pre/postamble.

    DMA queues (byte-fair round-robin across active rings):
      sync ring:   W1m0 -> W1m1   (gates L1, smallest-first)
      scalar ring: x1T -> W2 -> W3
      gpsimd SWDGE: x2T + biases  (only needed later, off the HWDGE rings)
    Engines:
      tensor: L1 -> L2 (interleaved psum groups) -> L3 (logvar chunk first)
      vector: x2 stats; relu(psum+b) per chunk; S1/T1 after ACT's iv/mu
      scalar: ACT-table preload dummies; tanh(lv) -> exp(+S0 accum) -> tanh(mu)
      gpsimd: T0 = sum iv*x2^2
      sync:   output DMA after all out_sb columns land; wait for completion
    """
    nc = bass.Bass()
    blob = nc.declare_dram_parameter("blob", [P, BLOB_W], DT, isOutput=False)
    out = nc.declare_dram_parameter("out", [P, 6], DT, isOutput=True)

    AF = mybir.ActivationFunctionType
    ALU = mybir.AluOpType

    with ExitStack() as ctx:
        ec = ctx.enter_context
        bsb = ec(nc.sbuf_tensor("bsb", [P, BLOB_W], DT))
        h00 = ec(nc.sbuf_tensor("h00", [P, ROWS], DT))
        h01 = ec(nc.sbuf_tensor("h01", [P, ROWS], DT))
        h10 = ec(nc.sbuf_tensor("h10", [P, ROWS], DT))
        h11 = ec(nc.sbuf_tensor("h11", [P, ROWS], DT))
        mu = ec(nc.sbuf_tensor("mu", [P, ROWS], DT))
        iv = ec(nc.sbuf_tensor("iv", [P, ROWS], DT))
        x2sq = ec(nc.sbuf_tensor("x2sq", [P, ROWS], DT))
        wmi = ec(nc.sbuf_tensor("wmi", [P, ROWS], DT))
        scr = ec(nc.sbuf_tensor("scr", [P, ROWS], DT))
        scr2 = ec(nc.sbuf_tensor("scr2", [P, ROWS], DT))
        out_sb = ec(nc.sbuf_tensor("out_sb", [P, 6], DT))
        ps0 = ec(nc.psum_tensor("ps0", [P, ROWS], DT))
        ps1 = ec(nc.psum_tensor("ps1", [P, ROWS], DT))
        ps2 = ec(nc.psum_tensor("ps2", [P, ROWS], DT))
        ps3 = ec(nc.psum_tensor("ps3", [P, ROWS], DT))
        ps4 = ec(nc.psum_tensor("ps4", [P, ROWS], DT))
        ps5 = ec(nc.psum_tensor("ps5", [P, ROWS], DT))
        psw = ec(nc.psum_tensor("psw", [P, ROWS], DT))
        dxa = ec(nc.semaphore("dxa"))
        dxb = ec(nc.semaphore("dxb"))
        dw1a = ec(nc.semaphore("dw1a"))
        dw1b = ec(nc.semaphore("dw1b"))
        dw2 = ec(nc.semaphore("dw2"))
        dw3 = ec(nc.semaphore("dw3"))
        s_pe = ec(nc.semaphore("s_pe"))
        s_act = ec(nc.semaphore("s_act"))
        s_dve = ec(nc.semaphore("s_dve"))
        s_gp = ec(nc.semaphore("s_gp"))
        dout = ec(nc.semaphore("dout"))
        block = ec(nc.Block())
        x1T = [bsb[:, 0:128], bsb[:, 128:256]]
        x2T = bsb[:, X2T_OFF : X2T_OFF + ROWS]

        # W section is m-major within each layer: col W_OFF + l*512 + m*256 + k*128
        def w_ap(l, k, m):
            c = W_OFF + l * 512 + m * 256 + k * 128
            return bsb[:, c : c + 128]

        def b_ap(l, m):
            c = B_OFF + 2 * l + m
            return bsb[:, c : c + 1]

        W1_OFF = W_OFF  # 390
        W2_OFF = W_OFF + 512  # 902
        W3_OFF = W_OFF + 1024  # 1414

        @block.sync
        def _(sync):
            sync.dma_start(
                out=bsb[:, W1_OFF : W1_OFF + 256], in_=blob[:, W1_OFF : W1_OFF + 256]
            ).then_inc(dw1a, 16)
            sync.dma_start(
                out=bsb[:, W1_OFF + 256 : W2_OFF], in_=blob[:, W1_OFF + 256 : W2_OFF]
            ).then_inc(dw1b, 16)
            sync.wait_ge(s_dve, 3)
            sync.dma_start(out=out[:], in_=out_sb[:]).then_inc(dout, 16)
            sync.wait_ge(dout, 16)

        @block.gpsimd
        def _(gpsimd):
            gpsimd.dma_start(
                out=bsb[:, 256:W1_OFF], in_=blob[:, 256:W1_OFF]
            ).then_inc(dxb, 16)

        @block.scalar
        def _(scalar):
            scalar.dma_start(out=bsb[:, 0:256], in_=blob[:, 0:256]).then_inc(
                dxa, 16
            )
            scalar.dma_start(
                out=bsb[:, W2_OFF:W3_OFF], in_=blob[:, W2_OFF:W3_OFF]
            ).then_inc(dw2, 16)
            scalar.dma_start(
                out=bsb[:, W3_OFF:BLOB_W], in_=blob[:, W3_OFF:BLOB_W]
            ).then_inc(dw3, 16)
            # dummy activations: pull the ACT table loads under the DMA shadow
            scalar.activation(
                out=scr[0:1, 0:1], in_=scr[0:1, 0:1], func=AF.Relu, scale=1.0
            )
            scalar.activation(
                out=scr[0:1, 0:1], in_=scr[0:1, 0:1], func=AF.Tanh, scale=1.0
            )
            scalar.activation(
                out=scr[0:1, 0:1], in_=scr[0:1, 0:1], func=AF.Exp, scale=0.0
            )
            scalar.wait_ge(dxb, 16)
            # odd-chunk relus run on ACT, in parallel with the even-chunk
            # relus on DVE
            scalar.wait_ge(s_pe, 2)
            scalar.activation(
                out=h01[:], in_=ps1[:], func=AF.Relu, bias=b_ap(0, 1), scale=1.0
            ).then_inc(s_act)
            scalar.wait_ge(s_pe, 3)
            scalar.activation(
                out=h10[:], in_=ps2[:], func=AF.Relu, bias=b_ap(1, 0), scale=1.0
            ).then_inc(s_act)
            # logvar chunk lands first (ps4): tanh -> exp(+S0 accum), then mu
            scalar.wait_ge(s_pe, 5)
            scalar.activation(
                out=iv[:], in_=ps4[:], func=AF.Tanh, bias=b_ap(2, 1), scale=1.0
            )
            scalar.activation(
                out=iv[:], in_=iv[:], func=AF.Exp, scale=-1.0,
                accum_out=out_sb[:, 0:1],
            ).then_inc(s_act)
            scalar.wait_ge(s_pe, 6)
            scalar.activation(
                out=mu[:], in_=ps5[:], func=AF.Tanh, bias=b_ap(2, 0), scale=1.0
            ).then_inc(s_act)

        @block.tensor
        def _(tensor):
            # Full-width dummy matmuls sized to end right as W1m0 lands: keeps
            # the PE HAM activity window hot CONTIGUOUSLY into the real MLP so
            # the clock-gate opens to 2.4 GHz from (close to) the start.
            for _i in range(10):
                tensor.matmul(psw[:], lhsT=bsb[:, 0:128], rhs=bsb[:, 0:128],
                              start=True, stop=True)
            tensor.wait_ge(dxa, 16)
            tensor.wait_ge(dw1a, 16)
            # L1 m0
            tensor.matmul(ps0[:], lhsT=w_ap(0, 0, 0), rhs=x1T[0], start=True, stop=False)
            tensor.matmul(ps0[:], lhsT=w_ap(0, 1, 0), rhs=x1T[1], start=False, stop=True).then_inc(s_pe)
            tensor.wait_ge(dw1b, 16)
            tensor.matmul(ps1[:], lhsT=w_ap(0, 0, 1), rhs=x1T[0], start=True, stop=False)
            tensor.matmul(ps1[:], lhsT=w_ap(0, 1, 1), rhs=x1T[1], start=False, stop=True).then_inc(s_pe)
            # L2: interleave the two psum groups so the k1 matmuls (which need
            # relu01) come as late as possible
            tensor.wait_ge(dw2, 16)
            tensor.wait_ge(s_dve, 1)
            tensor.matmul(ps2[:], lhsT=w_ap(1, 0, 0), rhs=h00[:], start=True, stop=False)
            tensor.matmul(ps3[:], lhsT=w_ap(1, 0, 1), rhs=h00[:], start=True, stop=False)
            tensor.wait_ge(s_act, 1)
            tensor.matmul(ps2[:], lhsT=w_ap(1, 1, 0), rhs=h01[:], start=False, stop=True).then_inc(s_pe)
            tensor.matmul(ps3[:], lhsT=w_ap(1, 1, 1), rhs=h01[:], start=False, stop=True).then_inc(s_pe)
            # L3 — logvar chunk (m=1) first so ACT can run tanh+exp while the
            # mu chunk is still on the PE
            tensor.wait_ge(dw3, 16)
            tensor.wait_ge(s_act, 2)
            tensor.matmul(ps4[:], lhsT=w_ap(2, 0, 1), rhs=h10[:], start=True, stop=False)
            tensor.wait_ge(s_dve, 2)
            tensor.matmul(ps4[:], lhsT=w_ap(2, 1, 1), rhs=h11[:], start=False, stop=True).then_inc(s_pe)
            tensor.matmul(ps5[:], lhsT=w_ap(2, 0, 0), rhs=h10[:], start=True, stop=False)
            tensor.matmul(ps5[:], lhsT=w_ap(2, 1, 0), rhs=h11[:], start=False, stop=True).then_inc(s_pe)

        @block.vector
        def _(vector):
            vector.wait_ge(dxb, 16)
            # even-chunk relus: out = max(psum + b, 0); odd chunks are on ACT
            vector.wait_ge(s_pe, 1)
            vector.tensor_scalar(
                out=h00[:], in0=ps0[:], scalar1=b_ap(0, 0), scalar2=0.0,
                op0=ALU.add, op1=ALU.max,
            ).then_inc(s_dve)
            vector.wait_ge(s_pe, 4)
            vector.tensor_scalar(
                out=h11[:], in0=ps3[:], scalar1=b_ap(1, 1), scalar2=0.0,
                op0=ALU.add, op1=ALU.max,
            ).then_inc(s_dve)
            # x2 stats fill the DVE idle window while the PE runs L2/L3
            vector.reduce_sum(
                out=out_sb[:, 2:3], in_=x2T, axis=mybir.AxisListType.X
            )
            vector.scalar_tensor_tensor(
                out=x2sq[:], in0=x2T, scalar=1.0, in1=x2T,
                op0=ALU.bypass, op1=ALU.mult, accum_out=out_sb[:, 3:4],
            )
            # T0 needs only iv (s_act>=3) and overlaps ACT's tanh(mu);
            # S1/T1 need mu too (s_act>=4)
            vector.wait_ge(s_act, 3)
            vector.scalar_tensor_tensor(
                out=scr2[:], in0=iv[:], scalar=1.0, in1=x2sq[:],
                op0=ALU.bypass, op1=ALU.mult, accum_out=out_sb[:, 4:5],
            )
            vector.wait_ge(s_act, 4)
            vector.scalar_tensor_tensor(
                out=wmi[:], in0=mu[:], scalar=1.0, in1=iv[:],
                op0=ALU.bypass, op1=ALU.mult, accum_out=out_sb[:, 1:2],
            )
            vector.scalar_tensor_tensor(
                out=scr[:], in0=wmi[:], scalar=1.0, in1=x2T,
                op0=ALU.bypass, op1=ALU.mult, accum_out=out_sb[:, 5:6],
            ).then_inc(s_dve)

    _split_multi_waits(nc)
    return nc


def _build_module_tile():
    nc = bass.Bass()
    blob = nc.declare_dram_parameter("blob", [P, BLOB_W], DT, isOutput=False)
    out = nc.declare_dram_parameter("out", [P, 6], DT, isOutput=True)

    AF = mybir.ActivationFunctionType
    ALU = mybir.AluOpType

    with tile.TileContext(nc) as tc:
        with (
            tc.tile_pool(name="sb", bufs=1) as sb,
            tc.tile_pool(name="ps", bufs=4, space="PSUM") as ps,
        ):
            bsb = sb.tile([P, BLOB_W], DT, tag="blob")
            nc.sync.dma_start(out=bsb[:], in_=blob[:])

            out_sb = sb.tile([P, 6], DT, tag="outsb")

            # This walrus build allows one sync-wait per compute instruction.
            # Touch the blob on ACT first so its engine clock observes the
            # input DMA; later activations then only wait on PE.
            warm = sb.tile([1, 1], DT, tag="warm")
            nc.scalar.copy(out=warm[:], in_=bsb[0:1, 0:1])

            x1T = [bsb[:, k * 128 : (k + 1) * 128] for k in range(2)]
            x2T = bsb[:, X2T_OFF : X2T_OFF + ROWS]

            def w_ap(l, k, m):
                c = W_OFF + l * 512 + k * 256 + m * 128
                return bsb[:, c : c + 128]

            def bias_ap(l, m):
                c = B_OFF + 2 * l + m
                return bsb[:, c : c + 1]

            # x2 shard stats: p1 = col-sums, p2 = col-sums of squares
            # (x2sq kept for T0 below)
            nc.vector.reduce_sum(
                out=out_sb[:, 2:3], in_=x2T, axis=mybir.AxisListType.X
            )
            x2sq = sb.tile([P, ROWS], DT, tag="x2sq")
            nc.vector.scalar_tensor_tensor(
                out=x2sq[:],
                in0=x2T,
                scalar=1.0,
                in1=x2T,
                op0=ALU.bypass,
                op1=ALU.mult,
                accum_out=out_sb[:, 3:4],
            )

            # MLP, feature-major: h_next[m] = act(sum_k W[k,m-slice].T @ h[k] + b[m])
            h = x1T
            for l in range(3):
                nxt = []
                for m in range(2):
                    pt = ps.tile([P, ROWS], DT, tag="mm")
                    for k in range(2):
                        nc.tensor.matmul(
                            pt[:],
                            lhsT=w_ap(l, k, m),
                            rhs=h[k],
                            start=(k == 0),
                            stop=(k == 1),
                        )
                    if l < 2:
                        hm = sb.tile([P, ROWS], DT, tag=f"h{l}{m}")
                        nc.scalar.activation(
                            out=hm[:],
                            in_=pt[:],
                            func=AF.Relu,
                            bias=bias_ap(l, m),
                            scale=1.0,
                        )
                        nxt.append(hm)
                    else:
                        nxt.append(pt)
                h = nxt

            mu = sb.tile([P, ROWS], DT, tag="mu")
            nc.scalar.activation(
                out=mu[:], in_=h[0][:], func=AF.Tanh, bias=bias_ap(2, 0), scale=1.0
            )
            lv = sb.tile([P, ROWS], DT, tag="lv")
            nc.scalar.activation(
                out=lv[:], in_=h[1][:], func=AF.Tanh, bias=bias_ap(2, 1), scale=1.0
            )
            iv = sb.tile([P, ROWS], DT, tag="iv")
            nc.scalar.activation(out=iv[:], in_=lv[:], func=AF.Exp, scale=-1.0)

            # All out_sb columns are written by DVE so the output DMA waits on
            # a single engine. S0 = sum iv:
            nc.vector.reduce_sum(
                out=out_sb[:, 0:1], in_=iv[:], axis=mybir.AxisListType.X
            )

            # wmi = mu*iv (accum S1), T0 = sum iv*x2^2, T1 = sum wmi*x2
            wmi = sb.tile([P, ROWS], DT, tag="wmi")
            nc.vector.scalar_tensor_tensor(
                out=wmi[:],
                in0=mu[:],
                scalar=1.0,
                in1=iv[:],
                op0=ALU.bypass,
                op1=ALU.mult,
                accum_out=out_sb[:, 1:2],
            )
            scr0 = sb.tile([P, ROWS], DT, tag="scr0")
            nc.vector.scalar_tensor_tensor(
                out=scr0[:],
                in0=iv[:],
                scalar=1.0,
                in1=x2sq[:],
                op0=ALU.bypass,
                op1=ALU.mult,
                accum_out=out_sb[:, 4:5],
            )
            scr1 = sb.tile([P, ROWS], DT, tag="scr1")
            nc.vector.scalar_tensor_tensor(
                out=scr1[:],
                in0=wmi[:],
                scalar=1.0,
                in1=x2T,
                op0=ALU.bypass,
                op1=ALU.mult,
                accum_out=out_sb[:, 5:6],
            )

            nc.sync.dma_start(out=out[:], in_=out_sb[:])
    _split_multi_waits(nc)
    return nc


def _split_multi_waits(nc):
    """This walrus build encodes at most one sync-wait per instruction.
    Hoist extra waits onto same-engine NoOps immediately preceding the
    instruction (engines execute their stream in order, so this is
    semantically identical)."""
    for fn in nc.m.functions:
        for bb in fn.blocks:
            new_insts = []
            for ins in bb.instructions:
                si = ins.sync_info
                if si is not None and len(si.on_wait) > 1:
                    waits = list(si.on_wait)
                    for j, w in enumerate(waits[:-1]):
                        nop = mybir.InstNoOp(
                            name=f"{ins.name}-sw{j}",
                            sync_info=mybir.SyncInfo(on_wait=[w], on_update=[]),
                            bass_nofuse=True,
                            engine=ins.engine,
                        )
                        new_insts.append(nop)
                    si.on_wait = [waits[-1]]
                new_insts.append(ins)
            if len(new_insts) != len(bb.instructions):
                bb.instructions[:] = new_insts


def _pack_inputs(x1, x2, W1, b1, W2, b2, W3, b3):
    f32 = np.float32
    wsec = np.empty((P, 3 * 512), f32)
    for l, W in enumerate((W1, W2, W3)):
        W = np.ascontiguousarray(W, f32)
        for m in range(2):
            for k in range(2):
                wsec[:, l * 512 + m * 256 + k * 128 : l * 512 + m * 256 + (k + 1) * 128] = W[
                    k * 128 : (k + 1) * 128, m * 128 : (m + 1) * 128
                ]
    in_maps = []
    for c in range(NCORES):
        blob = np.empty((P, BLOB_W), f32)
        x1s = np.asarray(x1[c * ROWS : (c + 1) * ROWS], f32)
        x2s = np.asarray(x2[c * ROWS : (c + 1) * ROWS], f32)
        blob[:, 0:128] = x1s[:, 0:128].T
        blob[:, 128:256] = x1s[:, 128:256].T
        blob[:, X2T_OFF : X2T_OFF + ROWS] = x2s.T
        for l, b in enumerate((b1, b2, b3)):
            b = np.asarray(b, f32)
            for m in range(2):
                blob[:, B_OFF + 2 * l + m] = b[m * 128 : (m + 1) * 128]
        blob[:, W_OFF:] = wsec
        in_maps.append({"blob": blob})
    return in_maps


def _run(in_maps, **kwargs):
    global _module_cache
    if _module_cache is None:
        _module_cache = _build_module()
    return run_bass_kernel_spmd(
        _module_cache, in_maps, core_ids=list(range(NCORES)), **kwargs
    )


def _combine(results):
    # cols: 0=S0, 1=S1, 2=p1, 3=p2, 4=T0, 5=T1
    acc = np.zeros((P, 6), np.float64)
    for r in results:
        acc += np.asarray(r["out"], np.float64)
    S0, S1, p1, p2, T0, T1 = (acc[:, i] for i in range(6))
    m1 = p1 / N
    m2 = p2 / N
    total = np.sum(-0.5 * T0 + 0.5 * m2 * S0 + T1 - m1 * S1)
    return np.float32(total / N)


def kernel(x1, x2, W1, b1, W2, b2, W3, b3):
    in_maps = _pack_inputs(x1, x2, W1, b1, W2, b2, W3, b3)
    res = _run(in_maps)
    return _combine(res.results)

